# revision 12
# baseline (speedup 1.0000x reference)
"""CrystalGraphEncoder (2x TransformerConv + 2x GATConv + LN + mean-pool + MLP)
as a Bass/Tile kernel on 8 Trainium2 NeuronCores.

Strategy: shard destination nodes across cores (edges sorted by dst). Per layer:
sharded dense matmuls -> fp16 gather tables -> AllGather (split in halves,
overlapped with the previous edge phase) -> bulk dma_gather of per-edge src
rows -> dst-side values broadcast to edges via transposed one-hot matmuls (no
gather) -> DVE edge math -> one-hot (fp8) scatter matmuls into PSUM ->
normalize + skip + residual + LN on-chip. The dense phase of layer L+1 is
interleaved into the edge loop of layer L. Mean-pool via one-hot matmul +
AllReduce; final MLP replicated.
"""
import numpy as np
import ml_dtypes

import concourse.bacc as bacc
import concourse.tile as tile
from concourse import bass, mybir
from concourse import bass_utils
from concourse.masks import make_identity

F16 = mybir.dt.float16
F32 = mybir.dt.float32
F8 = mybir.dt.float8e4
I16 = mybir.dt.int16
NP_F16 = np.float16
NP_F8 = ml_dtypes.float8_e4m3
P = 128

# problem constants (from the reference model)
N_NODES = 20000
IN_DIM = 92
HID = 256
OUT_DIM = 128
HEADS = 8
HDIM = 32
N_GRAPHS = 128
LN_EPS = 1e-5
C = 8  # cores
AF = mybir.ActivationFunctionType
PRELU = AF.Prelu  # sim_test overrides (CoreSim lacks Prelu)


def _wrap_idxs(idx):
    """[n] int -> [128, n//16] int16 dma_gather index layout (16-partition wrap,
    replicated for the 8 Q7 cores)."""
    n = len(idx)
    assert n % 16 == 0
    w = idx.reshape(n // 16, 16).T.astype(np.int16)
    return np.ascontiguousarray(np.tile(w, (8, 1)))


def _edge_struct(src_row, dst_local, dst_core, NB, pad_kv):
    """Per-core gather/scatter arrays for one edge set (sorted by dst)."""
    blk = dst_local // P
    slot = dst_local % P
    key = dst_core * NB + blk
    order = np.argsort(key, kind="stable")
    src_s = src_row[order]
    slot_s = slot[order]
    counts = np.bincount(key, minlength=C * NB)
    NT = int(np.ceil(counts.max() / P))
    starts = np.concatenate([[0], np.cumsum(counts)])

    per_core = []
    for c in range(C):
        n_slots = NB * NT * P
        kv_idx = np.full(n_slots, pad_kv, dtype=np.int64)
        S = np.zeros((NB, P, NT * P), dtype=NP_F8)
        for b in range(NB):
            k = c * NB + b
            s, e = starts[k], starts[k + 1]
            n = e - s
            pos = b * NT * P + np.arange(n)  # flat position j*128+p
            kv_idx[pos] = src_s[s:e]
            jj = np.arange(n) // P
            pp = np.arange(n) % P
            S[b, pp, jj * P + slot_s[s:e]] = 1.0
        # St: transpose of S within each (block, 128-col tile):
        # St[b, s, j*128+p] = S[b, p, j*128+s]
        S4 = S.reshape(NB, P, NT, P)
        St = np.ascontiguousarray(
            S4.transpose(0, 3, 2, 1).reshape(NB * P, NT * P)
        )
        per_core.append(
            dict(
                kv_idx=_wrap_idxs(kv_idx),
                S=np.ascontiguousarray(S.reshape(NB * P, NT * P)),
                St=St,
            )
        )
    return NT, per_core


def host_prep(inputs):
    """Split + pad + sort everything on the host. Returns (meta, in_maps)."""
    x = np.asarray(inputs["x"], np.float32)
    ei = np.asarray(inputs["edge_index"], np.int64)
    batch = np.asarray(inputs["batch"], np.int64)
    N = x.shape[0]
    RPC = (N + C - 1) // C
    NB = (RPC + P - 1) // P
    NPC = NB * P

    core_of = np.minimum(np.arange(N) // RPC, C - 1)
    local_of = np.arange(N) - core_of * RPC
    # Global-table row layout supports half-split AllGathers:
    # [half0: C chunks of NPC/2 rows][half1: C chunks of NPC/2 rows]
    if NB % 2 == 0:
        H = NPC // 2
        half = local_of // H
        grow = half * (C * H) + core_of * H + (local_of - half * H)
    else:
        grow = core_of * NPC + local_of

    PAD_KV = 0  # pad slots have zero one-hot weight; any finite row works

    src, dst = ei[0], ei[1]
    NTT, tconv = _edge_struct(grow[src], local_of[dst], core_of[dst], NB, PAD_KV)
    sl = np.arange(N, dtype=np.int64)
    src_g = np.concatenate([src, sl])
    dst_g = np.concatenate([dst, sl])
    NTG, gat = _edge_struct(grow[src_g], local_of[dst_g], core_of[dst_g], NB, PAD_KV)

    cnt = np.bincount(batch, minlength=N_GRAPHS).astype(np.float32)
    invcnt = (1.0 / np.maximum(cnt, 1.0)).reshape(N_GRAPHS, 1)

    def f16(a):
        return np.asarray(a, np.float32).astype(NP_F16)

    def pack_k(w):  # [K, N] -> [128, K//128 * N] (chunk-major)
        w = np.asarray(w, np.float32)
        K, Nc = w.shape
        assert K % P == 0
        return np.ascontiguousarray(
            w.reshape(K // P, P, Nc).transpose(1, 0, 2).reshape(P, -1)
        ).astype(NP_F16)

    wdict = dict(
        win=f16(inputs["Win"]),
        b_in=f16(np.asarray(inputs["b_in"]).reshape(1, HID)),
        w1=pack_k(inputs["W1"]),
        b1=f16(np.asarray(inputs["b1"]).reshape(1, 2 * HID)),
        w2=pack_k(inputs["W2"]),
        b2=f16(np.asarray(inputs["b2"]).reshape(1, OUT_DIM)),
        invcnt=invcnt.astype(np.float32),
    )
    for t in range(2):
        wkv = np.concatenate(
            [np.asarray(inputs["Wk"][t]), np.asarray(inputs["Wv"][t])], axis=1
        )
        bkv = np.concatenate(
            [np.asarray(inputs["bk"][t]), np.asarray(inputs["bv"][t])]
        )
        wdict[f"wkv{t}"] = pack_k(wkv)
        wdict[f"bkv{t}"] = f16(bkv.reshape(1, 2 * HID))
        wdict[f"wq{t}"] = pack_k(inputs["Wq"][t])
        wdict[f"bq{t}"] = f16(np.asarray(inputs["bq"][t]).reshape(1, HID))
        wdict[f"wsk{t}"] = pack_k(
            np.asarray(inputs["Wskip"][t], np.float64) + np.eye(HID)
        )
        wdict[f"bsk{t}"] = f16(np.asarray(inputs["bskip"][t]).reshape(1, HID))
        wdict[f"wg{t}"] = pack_k(inputs["Wg"][t])
        wdict[f"bgb{t}"] = np.ascontiguousarray(
            np.broadcast_to(
                f16(np.asarray(inputs["bg"][t]).reshape(1, HID)), (P, HID)
            )
        )
        wdict[f"atts{t}"] = np.ascontiguousarray(
            np.broadcast_to(
                f16(np.asarray(inputs["att_src"][t]).reshape(1, HID)), (P, HID)
            )
        )
        wdict[f"attd{t}"] = np.ascontiguousarray(
            np.broadcast_to(
                f16(np.asarray(inputs["att_dst"][t]).reshape(1, HID)), (P, HID)
            )
        )

    ln_g = np.asarray(inputs["ln_g"], np.float32)
    ln_b = np.asarray(inputs["ln_b"], np.float32)
    ln_trivial = bool(np.all(ln_g == 1.0) and np.all(ln_b == 0.0))
    if not ln_trivial:
        for i in range(4):
            wdict[f"lng{i}"] = np.ascontiguousarray(
                np.broadcast_to(ln_g[i].reshape(1, HID).astype(NP_F16), (P, HID))
            )
            wdict[f"lnb{i}"] = np.ascontiguousarray(
                np.broadcast_to(ln_b[i].reshape(1, HID).astype(NP_F16), (P, HID))
            )

    in_maps = []
    for c in range(C):
        m = dict(wdict)
        lo, hi = c * RPC, min((c + 1) * RPC, N)
        xT = np.zeros((IN_DIM, NPC), np.float32)
        xT[:, 0 : hi - lo] = x[lo:hi].T
        m["xT"] = xT.astype(NP_F16)
        m["kvidx"] = tconv[c]["kv_idx"]
        m["S_t"] = tconv[c]["S"]
        m["St_t"] = tconv[c]["St"]
        m["gatidx"] = gat[c]["kv_idx"]
        m["S_g"] = gat[c]["S"]
        m["St_g"] = gat[c]["St"]
        Sp = np.zeros((NB, P, N_GRAPHS), dtype=NP_F8)
        ns = hi - lo
        bb = np.arange(ns) // P
        pp = np.arange(ns) % P
        Sp[bb, pp, batch[lo:hi]] = 1.0
        m["S_p"] = np.ascontiguousarray(Sp.reshape(NB * P, N_GRAPHS))
        in_maps.append(m)

    meta = dict(NB=NB, NPC=NPC, NTT=NTT, NTG=NTG, ln_trivial=ln_trivial)
    return meta, in_maps


def build_program(meta):
    NB = meta["NB"]
    NPC = meta["NPC"]
    NTT = meta["NTT"]
    NTG = meta["NTG"]
    ln_trivial = meta["ln_trivial"]
    TABN = C * NPC
    NTMAX = max(NTT, NTG)
    QGRP = 2  # q-broadcast tiles per PSUM group (1 f32 PSUM bank)
    SPLIT_AG = NB % 2 == 0
    AG_TRIG = NB // 2 + 2  # edge-loop block after which half-0 AG is issued

    nc = bacc.Bacc("TRN2", target_bir_lowering=False, debug=False, num_devices=C)

    def di(name, shape, dt):
        return nc.dram_tensor(name, shape, dt, kind="ExternalInput")

    xT_d = di("xT", [IN_DIM, NPC], F16)
    kvidx_d = di("kvidx", [P, NB * NTT * 8], I16)
    St_d = di("S_t", [NB * P, NTT * P], F8)
    Stt_d = di("St_t", [NB * P, NTT * P], F8)
    gatidx_d = di("gatidx", [P, NB * NTG * 8], I16)
    Sg_d = di("S_g", [NB * P, NTG * P], F8)
    Stg_d = di("St_g", [NB * P, NTG * P], F8)
    Sp_d = di("S_p", [NB * P, N_GRAPHS], F8)
    invcnt_d = di("invcnt", [N_GRAPHS, 1], F32)
    win_d = di("win", [IN_DIM, HID], F16)
    bin_d = di("b_in", [1, HID], F16)
    w1_d = di("w1", [P, 2 * 2 * HID], F16)
    b1_d = di("b1", [1, 2 * HID], F16)
    w2_d = di("w2", [P, 4 * OUT_DIM], F16)
    b2_d = di("b2", [1, OUT_DIM], F16)
    wd = {}
    for t in range(2):
        wd[f"wkv{t}"] = di(f"wkv{t}", [P, 2 * 2 * HID], F16)
        wd[f"bkv{t}"] = di(f"bkv{t}", [1, 2 * HID], F16)
        wd[f"wq{t}"] = di(f"wq{t}", [P, 2 * HID], F16)
        wd[f"bq{t}"] = di(f"bq{t}", [1, HID], F16)
        wd[f"wsk{t}"] = di(f"wsk{t}", [P, 2 * HID], F16)
        wd[f"bsk{t}"] = di(f"bsk{t}", [1, HID], F16)
        wd[f"wg{t}"] = di(f"wg{t}", [P, 2 * HID], F16)
        wd[f"bgb{t}"] = di(f"bgb{t}", [P, HID], F16)
        wd[f"atts{t}"] = di(f"atts{t}", [P, HID], F16)
        wd[f"attd{t}"] = di(f"attd{t}", [P, HID], F16)
    if not ln_trivial:
        for i in range(4):
            wd[f"lng{i}"] = di(f"lng{i}", [P, HID], F16)
            wd[f"lnb{i}"] = di(f"lnb{i}", [P, HID], F16)

    out_d = nc.dram_tensor("out", [N_GRAPHS, OUT_DIM], F32, kind="ExternalOutput")

    h_all = nc.alloc_sbuf_tensor("h_all", [P, NB * HID], F32)
    hT_all = nc.alloc_sbuf_tensor("hT_all", [P, 2 * NPC], F16)
    q_all = nc.alloc_sbuf_tensor("q_all", [P, NB * HID], F16)
    skip_all = nc.alloc_sbuf_tensor("skip_all", [P, NB * HID], F16)
    ad_all = nc.alloc_sbuf_tensor("ad_all", [P, NB * HEADS], F16)
    xT_sb = nc.alloc_sbuf_tensor("xT_sb", [IN_DIM, NPC], F16)
    kvidx_sb = nc.alloc_sbuf_tensor("kvidx_sb", [P, NB * NTT * 8], I16)
    gatidx_sb = nc.alloc_sbuf_tensor("gatidx_sb", [P, NB * NTG * 8], I16)

    SQ32 = 1.0 / float(np.sqrt(HDIM))

    with tile.TileContext(nc) as tc:
        with (
            tc.tile_pool(name="wpool", bufs=1) as wp,
            tc.tile_pool(name="spool", bufs=2) as sp,
            tc.tile_pool(name="gpool", bufs=2) as gp,
            tc.tile_pool(name="psA", bufs=2, space="PSUM") as psA,
            tc.tile_pool(name="psQ", bufs=2, space="PSUM") as psQ,
            tc.tile_pool(name="psT", bufs=1, space="PSUM") as psT,
            tc.tile_pool(name="psC", bufs=1, space="PSUM") as psC,
            tc.tile_pool(name="dram", bufs=1, space="DRAM") as dp,
        ):
            nc.sync.dma_start(xT_sb.ap(), xT_d.ap())
            nc.sync.dma_start(kvidx_sb.ap(), kvidx_d.ap())
            nc.sync.dma_start(gatidx_sb.ap(), gatidx_d.ap())

            ident = wp.tile([P, P], F16, tag="ident")
            make_identity(nc, ident[:])
            ones1 = wp.tile([1, P], F16, tag="ones1")
            nc.vector.memset(ones1[:], 1.0)

            def load_w(d, shape, tag, dt=F16):
                t = wp.tile(shape, dt, tag=tag)
                nc.sync.dma_start(t[:], d.ap())
                return t

            win_t = load_w(win_d, [IN_DIM, HID], "win")
            bin_t = load_w(bin_d, [1, HID], "b_in")
            w1_t = load_w(w1_d, [P, 2 * 2 * HID], "w1")
            b1_t = load_w(b1_d, [1, 2 * HID], "b1")
            w2_t = load_w(w2_d, [P, 4 * OUT_DIM], "w2")
            b2_t = load_w(b2_d, [1, OUT_DIM], "b2")
            wt = {}
            for t in range(2):
                for nm, sh in [
                    (f"wkv{t}", [P, 2 * 2 * HID]),
                    (f"bkv{t}", [1, 2 * HID]),
                    (f"wq{t}", [P, 2 * HID]),
                    (f"bq{t}", [1, HID]),
                    (f"wsk{t}", [P, 2 * HID]),
                    (f"bsk{t}", [1, HID]),
                    (f"wg{t}", [P, 2 * HID]),
                    (f"bgb{t}", [P, HID]),
                    (f"atts{t}", [P, HID]),
                    (f"attd{t}", [P, HID]),
                ]:
                    wt[nm] = load_w(wd[nm], sh, nm)
            if not ln_trivial:
                for i in range(4):
                    wt[f"lng{i}"] = load_w(wd[f"lng{i}"], [P, HID], f"lng{i}")
                    wt[f"lnb{i}"] = load_w(wd[f"lnb{i}"], [P, HID], f"lnb{i}")
            invcnt_t = load_w(invcnt_d, [N_GRAPHS, 1], "invcnt", F32)
            Sp_sb = []
            for b in range(NB):
                spt = wp.tile([P, N_GRAPHS], F8, tag=f"S_p{b}")
                nc.sync.dma_start(spt[:], Sp_d.ap()[b * P : (b + 1) * P, :])
                Sp_sb.append(spt)

            def mm_dense(psum, lhsT0, lhsT1, w_tile, ncols, bias_tile):
                nc.tensor.matmul(
                    psum, lhsT=lhsT0, rhs=w_tile[:, 0:ncols], start=True, stop=False
                )
                nc.tensor.matmul(
                    psum, lhsT=lhsT1, rhs=w_tile[:, ncols : 2 * ncols],
                    start=False, stop=False,
                )
                nc.tensor.matmul(
                    psum, lhsT=ones1[:], rhs=bias_tile[:, 0:ncols],
                    start=False, stop=True,
                )

            def hT_slices(b):
                l0 = hT_all.ap()[:, 0 * NPC + b * P : 0 * NPC + (b + 1) * P]
                l1 = hT_all.ap()[:, 1 * NPC + b * P : 1 * NPC + (b + 1) * P]
                return l0, l1

            def store_h_and_hT(src_sb_f32, b):
                h16 = sp.tile([P, HID], F16, tag="h16")
                nc.scalar.activation(h16[:], src_sb_f32, AF.Copy)
                for f in range(2):
                    ptp = psT.tile([P, P], F16, space="PSUM", tag="ptp")
                    nc.tensor.transpose(ptp[:], h16[:, f * P : (f + 1) * P], ident[:])
                    nc.scalar.activation(
                        hT_all.ap()[:, f * NPC + b * P : f * NPC + (b + 1) * P],
                        ptp[:],
                        AF.Copy,
                    )
                return h16

            kv_tabs = [
                dp.tile([TABN, 2 * HID], F16, tag=f"kv_tab{t}", name=f"kv_tab{t}")
                for t in range(2)
            ]
            gat_tabs = [
                dp.tile([TABN, 384], F16, tag=f"gat_tab{t}", name=f"gat_tab{t}")
                for t in range(2)
            ]
            kv_bnc = dp.tile([NPC, 2 * HID], F16, tag="kv_bnc")
            gat_bnc = dp.tile([NPC, 384], F16, tag="gat_bnc")
            pool_in = dp.tile([N_GRAPHS, HID], F32, tag="pool_in")
            pool_out = dp.tile([N_GRAPHS, HID], F32, tag="pool_out")

            psum_pool = psC.tile([N_GRAPHS, HID], F32, space="PSUM", tag="ps_pool")

            def dense_block(layer, b):
                """Dense projections of layer `layer` for block b (reads hT)."""
                is_t = layer % 2 == 0
                t = layer // 2
                l0, l1 = hT_slices(b)
                rows = slice(b * P, (b + 1) * P)
                hcols = slice(b * HID, (b + 1) * HID)
                if is_t:
                    ps = psA.tile([P, 2 * HID], F32, space="PSUM", tag="ps_dense")
                    mm_dense(ps[:], l0, l1, wt[f"wkv{t}"], 2 * HID, wt[f"bkv{t}"])
                    kv16 = sp.tile([P, 2 * HID], F16, tag="kv16")
                    nc.scalar.activation(kv16[:], ps[:], AF.Copy)
                    nc.sync.dma_start(kv_bnc[rows, :], kv16[:])
                    ps2 = psQ.tile([P, HID], F32, space="PSUM", tag="psq")
                    mm_dense(ps2[:], l0, l1, wt[f"wq{t}"], HID, wt[f"bq{t}"])
                    nc.scalar.activation(q_all.ap()[:, hcols], ps2[:], AF.Copy)
                    ps3 = psQ.tile([P, HID], F32, space="PSUM", tag="psq")
                    mm_dense(ps3[:], l0, l1, wt[f"wsk{t}"], HID, wt[f"bsk{t}"])
                    nc.scalar.activation(skip_all.ap()[:, hcols], ps3[:], AF.Copy)
                else:
                    ps = psA.tile([P, 2 * HID], F32, space="PSUM", tag="ps_dense")
                    nc.tensor.matmul(
                        ps[:, 0:HID], lhsT=l0, rhs=wt[f"wg{t}"][:, 0:HID],
                        start=True, stop=False,
                    )
                    nc.tensor.matmul(
                        ps[:, 0:HID], lhsT=l1, rhs=wt[f"wg{t}"][:, HID : 2 * HID],
                        start=False, stop=True,
                    )
                    hh16 = sp.tile([P, HID], F16, tag="hh16")
                    nc.scalar.activation(hh16[:], ps[:, 0:HID], AF.Copy)
                    nc.sync.dma_start(gat_bnc[rows, 0:HID], hh16[:])
                    for which, wnm in ((0, f"atts{t}"), (1, f"attd{t}")):
                        proda = sp.tile([P, HID], F16, tag="prodA")
                        nc.vector.tensor_tensor(
                            out=proda[:], in0=hh16[:], in1=wt[wnm][:],
                            op=mybir.AluOpType.mult,
                        )
                        asum = sp.tile([P, HEADS], F32, tag="asum")
                        nc.vector.tensor_reduce(
                            out=asum[:],
                            in_=proda[:].rearrange("p (h w) -> p h w", h=HEADS),
                            axis=mybir.AxisListType.X,
                            op=mybir.AluOpType.add,
                        )
                        if which == 0:
                            a16 = sp.tile([P, HEADS], F16, tag="a16")
                            nc.scalar.activation(a16[:], asum[:], AF.Copy)
                            nc.sync.dma_start(
                                gat_bnc[rows, HID : HID + HEADS], a16[:]
                            )
                        else:
                            nc.vector.tensor_scalar(
                                out=ad_all.ap()[:, b * HEADS : (b + 1) * HEADS],
                                in0=asum[:],
                                scalar1=1.0,
                                scalar2=None,
                                op0=mybir.AluOpType.mult,
                            )

            def issue_ag(layer, half):
                """AllGather (one half if SPLIT_AG) of this layer's table."""
                is_t = layer % 2 == 0
                t = layer // 2
                tab = kv_tabs[t] if is_t else gat_tabs[t]
                bnc = kv_bnc if is_t else gat_bnc
                if SPLIT_AG:
                    H = NPC // 2
                    CH = C * H
                    ins = [bnc[half * H : (half + 1) * H, :].opt()]
                    outs = [tab[half * CH : (half + 1) * CH, :]]
                else:
                    if half == 1:
                        return
                    ins = [bnc.opt()]
                    outs = [tab[0:TABN, :]]
                nc.gpsimd.collective_compute(
                    "AllGather",
                    mybir.AluOpType.bypass,
                    replica_groups=[list(range(C))],
                    ins=ins,
                    outs=outs,
                )

            def edge_block(layer, b):
                is_t = layer % 2 == 0
                t = layer // 2
                NT = NTT if is_t else NTG
                NE = NT * P
                isl = slice(b * NT * 8, (b + 1) * NT * 8)
                hcols = slice(b * HID, (b + 1) * HID)
                if is_t:
                    g_kv = gp.tile([P, NTMAX, 2 * HID], F16, tag="g_big")
                    nc.gpsimd.dma_gather(
                        g_kv[:, 0:NT, :], kv_tabs[t][:], kvidx_sb.ap()[:, isl],
                        NE, NE, 2 * HID, single_packet=False,
                    )
                    S_sb = gp.tile([P, NTMAX * P], F8, tag="S_sb")
                    nc.sync.dma_start(
                        S_sb[:, 0 : NT * P], St_d.ap()[b * P : (b + 1) * P, :]
                    )
                    St_sb = gp.tile([P, NTMAX * P], F8, tag="St_sb")
                    nc.sync.dma_start(
                        St_sb[:, 0 : NT * P], Stt_d.ap()[b * P : (b + 1) * P, :]
                    )
                    vpart = g_kv[:, 0:NT, HID : 2 * HID]
                else:
                    g_kv = gp.tile([P, NTMAX, 384], F16, tag="g_big")
                    nc.gpsimd.dma_gather(
                        g_kv[:, 0:NT, :], gat_tabs[t][:], gatidx_sb.ap()[:, isl],
                        NE, NE, 384, single_packet=False,
                    )
                    S_sb = gp.tile([P, NTMAX * P], F8, tag="S_sb")
                    nc.sync.dma_start(
                        S_sb[:, 0 : NT * P], Sg_d.ap()[b * P : (b + 1) * P, :]
                    )
                    St_sb = gp.tile([P, NTMAX * P], F8, tag="St_sb")
                    nc.sync.dma_start(
                        St_sb[:, 0 : NT * P], Stg_d.ap()[b * P : (b + 1) * P, :]
                    )
                    vpart = g_kv[:, 0:NT, 0:HID]

                rhs = gp.tile([P, NTMAX, HID + HEADS], F16, tag="rhs")
                red = gp.tile([P, NTMAX * HEADS], F32, tag="red")
                expdst = rhs[:, 0:NT, HID : HID + HEADS]
                if is_t:
                    # per-edge q rows via transposed one-hot matmuls,
                    # in groups of QGRP tiles per PSUM bank
                    for g0 in range(0, NT, QGRP):
                        gsz = min(QGRP, NT - g0)
                        psq = psQ.tile(
                            [P, QGRP * HID], F32, space="PSUM", tag="psq"
                        )
                        for jj in range(gsz):
                            j = g0 + jj
                            nc.tensor.matmul(
                                psq[:, jj * HID : (jj + 1) * HID],
                                lhsT=St_sb[:, j * P : (j + 1) * P],
                                rhs=q_all.ap()[:, hcols],
                                start=True,
                                stop=True,
                            )
                        qk = sp.tile([P, QGRP * HID], F16, tag="qk")
                        nc.vector.tensor_tensor(
                            out=qk[:, 0 : gsz * HID].rearrange(
                                "p (t f) -> p t f", t=gsz
                            ),
                            in0=g_kv[:, g0 : g0 + gsz, 0:HID],
                            in1=psq[:, 0 : gsz * HID].rearrange(
                                "p (t f) -> p t f", t=gsz
                            ),
                            op=mybir.AluOpType.mult,
                        )
                        nc.vector.tensor_reduce(
                            out=red[:, g0 * HEADS : (g0 + gsz) * HEADS],
                            in_=qk[:, 0 : gsz * HID].rearrange(
                                "p (t h w) -> p t h w", t=gsz, h=HEADS
                            ),
                            axis=mybir.AxisListType.X,
                            op=mybir.AluOpType.add,
                        )
                    nc.scalar.activation(
                        expdst,
                        red[:, 0 : NT * HEADS].rearrange(
                            "p (t h) -> p t h", h=HEADS
                        ),
                        AF.Exp,
                        scale=SQ32,
                    )
                else:
                    # per-edge a_dst via transposed one-hot matmuls into
                    # one narrow PSUM tile
                    psad = psQ.tile(
                        [P, NTMAX * HEADS], F32, space="PSUM", tag="psq"
                    )
                    for j in range(NT):
                        nc.tensor.matmul(
                            psad[:, j * HEADS : (j + 1) * HEADS],
                            lhsT=St_sb[:, j * P : (j + 1) * P],
                            rhs=ad_all.ap()[:, b * HEADS : (b + 1) * HEADS],
                            start=True,
                            stop=True,
                        )
                    esum = gp.tile([P, NTMAX * HEADS], F16, tag="esum")
                    nc.vector.tensor_tensor(
                        out=esum[:, 0 : NT * HEADS].rearrange(
                            "p (t h) -> p t h", h=HEADS
                        ),
                        in0=g_kv[:, 0:NT, HID : HID + HEADS],
                        in1=psad[:, 0 : NT * HEADS].rearrange(
                            "p (t h) -> p t h", h=HEADS
                        ),
                        op=mybir.AluOpType.add,
                    )
                    nc.scalar.activation(
                        red[:, 0 : NT * HEADS],
                        esum[:, 0 : NT * HEADS],
                        PRELU,
                        alpha=0.2,
                    )
                    nc.scalar.activation(
                        expdst,
                        red[:, 0 : NT * HEADS].rearrange(
                            "p (t h) -> p t h", h=HEADS
                        ),
                        AF.Exp,
                    )
                nc.vector.tensor_tensor(
                    out=rhs[:, 0:NT, 0:HID].rearrange(
                        "p t (h w) -> p t h w", h=HEADS
                    ),
                    in0=vpart.rearrange("p t (h w) -> p t h w", h=HEADS),
                    in1=expdst.to_broadcast([P, NT, HEADS, HDIM]),
                    op=mybir.AluOpType.mult,
                )
                ps_agg = psA.tile([P, HID + HEADS], F32, space="PSUM", tag="ps_agg")
                for j in range(NT):
                    nc.tensor.matmul(
                        ps_agg[:],
                        lhsT=S_sb[:, j * P : (j + 1) * P],
                        rhs=rhs[:, j, :],
                        start=(j == 0),
                        stop=(j == NT - 1),
                    )
                den = sp.tile([P, HEADS], F32, tag="den")
                nc.scalar.activation(
                    den[:], ps_agg[:, HID : HID + HEADS], AF.Copy, bias=1e-16
                )
                rec = sp.tile([P, HEADS], F32, tag="rec")
                nc.vector.reciprocal(rec[:], den[:])
                t1 = sp.tile([P, HID], F32, tag="t1")
                nc.vector.tensor_tensor(
                    out=t1[:].rearrange("p (h w) -> p h w", h=HEADS),
                    in0=ps_agg[:, 0:HID].rearrange("p (h w) -> p h w", h=HEADS),
                    in1=rec[:].to_broadcast([P, HEADS, HDIM]),
                    op=mybir.AluOpType.mult,
                )
                # t2 = t1 + skip (+ res for GAT)
                t2 = sp.tile([P, HID], F32, tag="t2")
                if is_t:
                    nc.vector.tensor_tensor(
                        out=t2[:], in0=t1[:],
                        in1=skip_all.ap()[:, hcols],
                        op=mybir.AluOpType.add,
                    )
                else:
                    t2a = sp.tile([P, HID], F32, tag="t2a")
                    nc.vector.tensor_tensor(
                        out=t2a[:], in0=t1[:],
                        in1=h_all.ap()[:, hcols],
                        op=mybir.AluOpType.add,
                    )
                    nc.vector.tensor_tensor(
                        out=t2[:], in0=t2a[:], in1=wt[f"bgb{t}"][:],
                        op=mybir.AluOpType.add,
                    )
                # LN via variance identity: var = E[x^2] - mu^2
                mus = sp.tile([P, 1], F32, tag="mus")
                nc.vector.reduce_sum(mus[:], t2[:], axis=mybir.AxisListType.X)
                sq = sp.tile([P, HID], F32, tag="sq")
                nc.scalar.activation(sq[:], t2[:], AF.Square)
                s2 = sp.tile([P, 1], F32, tag="s2")
                nc.vector.reduce_sum(s2[:], sq[:], axis=mybir.AxisListType.X)
                mu = sp.tile([P, 1], F32, tag="mu")
                nc.vector.tensor_scalar(
                    out=mu[:], in0=mus[:], scalar1=1.0 / HID, scalar2=None,
                    op0=mybir.AluOpType.mult,
                )
                mu2 = sp.tile([P, 1], F32, tag="mu2")
                nc.vector.tensor_tensor(
                    out=mu2[:], in0=mu[:], in1=mu[:], op=mybir.AluOpType.mult
                )
                bias_v = sp.tile([P, 1], F32, tag="bias_v")
                nc.vector.tensor_scalar(
                    out=bias_v[:], in0=mu2[:], scalar1=-1.0,
                    scalar2=LN_EPS, op0=mybir.AluOpType.mult,
                    op1=mybir.AluOpType.add,
                )
                sd = sp.tile([P, 1], F32, tag="sd")
                nc.scalar.activation(
                    sd[:], s2[:], AF.Sqrt, scale=1.0 / HID, bias=bias_v[:, 0:1]
                )
                rs = sp.tile([P, 1], F32, tag="rs")
                nc.vector.reciprocal(rs[:], sd[:])
                bias2 = sp.tile([P, 1], F32, tag="bias2")
                nc.vector.tensor_tensor(
                    out=bias2[:], in0=mu[:], in1=rs[:], op=mybir.AluOpType.mult
                )
                nc.vector.tensor_scalar(
                    out=bias2[:], in0=bias2[:], scalar1=-1.0, scalar2=None,
                    op0=mybir.AluOpType.mult,
                )
                hdst = h_all.ap()[:, hcols]
                if ln_trivial:
                    nc.scalar.activation(
                        hdst, t2[:], AF.Relu, scale=rs[:, 0:1], bias=bias2[:, 0:1]
                    )
                else:
                    t3 = sp.tile([P, HID], F32, tag="t3")
                    nc.vector.tensor_scalar(
                        out=t3[:], in0=t2[:], scalar1=rs[:, 0:1],
                        scalar2=bias2[:, 0:1], op0=mybir.AluOpType.mult,
                        op1=mybir.AluOpType.add,
                    )
                    nc.vector.tensor_tensor(
                        out=t3[:], in0=t3[:], in1=wt[f"lng{layer}"][:],
                        op=mybir.AluOpType.mult,
                    )
                    nc.vector.tensor_tensor(
                        out=t3[:], in0=t3[:], in1=wt[f"lnb{layer}"][:],
                        op=mybir.AluOpType.add,
                    )
                    nc.scalar.activation(hdst, t3[:], AF.Relu)
                h16 = store_h_and_hT(hdst, b)
                if layer == 3:
                    nc.tensor.matmul(
                        psum_pool[:],
                        lhsT=Sp_sb[b][:],
                        rhs=h16[:],
                        start=(b == 0),
                        stop=(b == NB - 1),
                    )

            # phase 0: h0 = x @ Win + b_in, with layer-0 dense interleaved
            for b in range(NB):
                ps = psA.tile([P, 2 * HID], F32, space="PSUM", tag="ps_dense")
                nc.tensor.matmul(
                    ps[:, 0:HID], lhsT=xT_sb.ap()[:, b * P : (b + 1) * P],
                    rhs=win_t[:], start=True, stop=False,
                )
                nc.tensor.matmul(
                    ps[:, 0:HID], lhsT=ones1[:], rhs=bin_t[:], start=False, stop=True
                )
                nc.scalar.activation(
                    h_all.ap()[:, b * HID : (b + 1) * HID], ps[:, 0:HID], AF.Copy
                )
                store_h_and_hT(h_all.ap()[:, b * HID : (b + 1) * HID], b)
                dense_block(0, b)
            issue_ag(0, 0)
            issue_ag(0, 1)

            # layer pipeline: edge phase of L interleaved with dense of L+1
            for layer in range(4):
                for b in range(NB):
                    edge_block(layer, b)
                    if layer < 3:
                        dense_block(layer + 1, b)
                        if b == AG_TRIG:
                            issue_ag(layer + 1, 0)
                if layer < 3:
                    issue_ag(layer + 1, 1)

            # ---- pool + MLP ----
            pool_sb = sp.tile([N_GRAPHS, HID], F32, tag="pool_sb")
            nc.scalar.activation(pool_sb[:], psum_pool[:], AF.Copy)
            nc.sync.dma_start(pool_in[:], pool_sb[:])
            nc.gpsimd.collective_compute(
                "AllReduce",
                mybir.AluOpType.add,
                replica_groups=[list(range(C))],
                ins=[pool_in.opt()],
                outs=[pool_out.opt()],
            )
            sums = sp.tile([N_GRAPHS, HID], F32, tag="sums")
            nc.sync.dma_start(sums[:], pool_out[:])
            pooled = sp.tile([N_GRAPHS, HID], F32, tag="pooled")
            nc.vector.tensor_scalar(
                out=pooled[:], in0=sums[:], scalar1=invcnt_t[:, 0:1],
                scalar2=None, op0=mybir.AluOpType.mult,
            )
            p16 = sp.tile([N_GRAPHS, HID], F16, tag="p16")
            nc.scalar.activation(p16[:], pooled[:], AF.Copy)
            pT = sp.tile([P, 2 * N_GRAPHS], F16, tag="pT")
            for f in range(2):
                ptp = psT.tile([P, P], F16, space="PSUM", tag="ptp")
                nc.tensor.transpose(ptp[:], p16[:, f * P : (f + 1) * P], ident[:])
                nc.scalar.activation(
                    pT[:, f * N_GRAPHS : (f + 1) * N_GRAPHS], ptp[:], AF.Copy
                )
            ps1 = psA.tile([P, 2 * HID], F32, space="PSUM", tag="ps_dense")
            nc.tensor.matmul(
                ps1[:], lhsT=pT[:, 0:N_GRAPHS], rhs=w1_t[:, 0 : 2 * HID],
                start=True, stop=False,
            )
            nc.tensor.matmul(
                ps1[:], lhsT=pT[:, N_GRAPHS : 2 * N_GRAPHS],
                rhs=w1_t[:, 2 * HID : 4 * HID], start=False, stop=False,
            )
            nc.tensor.matmul(
                ps1[:], lhsT=ones1[:], rhs=b1_t[:], start=False, stop=True
            )
            h1 = sp.tile([N_GRAPHS, 2 * HID], F16, tag="h1")
            nc.scalar.activation(h1[:], ps1[:], AF.Relu)
            h1T = sp.tile([P, 4 * N_GRAPHS], F16, tag="h1T")
            for f in range(4):
                ptp = psT.tile([P, P], F16, space="PSUM", tag="ptp")
                nc.tensor.transpose(ptp[:], h1[:, f * P : (f + 1) * P], ident[:])
                nc.scalar.activation(
                    h1T[:, f * N_GRAPHS : (f + 1) * N_GRAPHS], ptp[:], AF.Copy
                )
            ps2 = psA.tile([P, 2 * HID], F32, space="PSUM", tag="ps_dense")
            for f in range(4):
                nc.tensor.matmul(
                    ps2[:, 0:OUT_DIM],
                    lhsT=h1T[:, f * N_GRAPHS : (f + 1) * N_GRAPHS],
                    rhs=w2_t[:, f * OUT_DIM : (f + 1) * OUT_DIM],
                    start=(f == 0),
                    stop=False,
                )
            nc.tensor.matmul(
                ps2[:, 0:OUT_DIM], lhsT=ones1[:], rhs=b2_t[:], start=False, stop=True
            )
            out_sb = sp.tile([N_GRAPHS, OUT_DIM], F32, tag="out_sb")
            nc.scalar.activation(out_sb[:], ps2[:, 0:OUT_DIM], AF.Copy)
            nc.sync.dma_start(out_d.ap(), out_sb[:])

    nc.compile()
    return nc


_CACHE = {}


def kernel(**inputs):
    meta, in_maps = host_prep(inputs)
    key = tuple(sorted(meta.items()))
    if key not in _CACHE:
        _CACHE[key] = build_program(meta)
    nc = _CACHE[key]
    res = bass_utils.run_bass_kernel_spmd(nc, in_maps, core_ids=list(range(C)))
    return np.asarray(res.results[0]["out"], np.float32)
